# revision 6
# baseline (speedup 1.0000x reference)
"""Trainium2 Bass kernel for causal multi-head attention (B=4, N=2048, DIM=1024, H=16, DH=64).

Sharding: 8 cores = (batch, head-group) pairs. Core c handles batch c//2 and
heads (c%2)*8 .. (c%2)*8+7.  Each core computes QKV projection for its 8 heads,
causal flash-attention, and a partial output projection (its heads' rows of
w_out).  The host sums the two partial outputs per batch and adds b_out.

Device-side layout choices (per core):
  - x is fed pre-transposed as xT [DIM, N] bf16 (host prep), so the QKV
    projection contraction (over DIM) sits on partitions with no on-device
    transpose.
  - Q^T, K^T computed as [head_dim, tok] (weights-stationary matmuls) so that
    scores can be computed directly as S^T = K^T.T @ Q^T with contraction dh=64.
  - S^T tiles are [128 k-tok, 512 q-tok]; softmax denominator comes free by
    augmenting V with a ones column: O^T_aug = [V | 1].T @ exp(S^T).
  - V computed as [tok, dh] (x-stationary matmuls), stored interleaved with the
    ones column: per k-tile [128, 8*65].
  - Causal masking: multiply exp(S^T) by precomputed 0/1 bf16 tiles on the
    diagonal blocks only (exp of a finite garbage score times 0 is exactly 0).
  - Key-padding mask folds into V_aug: V_aug row k scaled by mask[k] zeroes both
    numerator and denominator contributions of masked keys.

Schedule (the performance-critical part): the PE is the bottleneck engine
(matmul stream floor ~225us vs ~296us measured for the phase-sequential
version).  The attend inner loop is software-pipelined (scores of k-pair kp+1
issue before PV of kp, so the PE does not sit directly behind the Scalar
engine's exp), and projection / out-projection matmuls are woven one-at-a-time
into the attend loops as "fillers" that absorb the exp-latency stalls.  The
last head pair interleaves heads per q-chunk so out-projection tiles become
ready progressively and the tail drains with the PE dense.
"""

import numpy as np
import ml_dtypes

B, N, DIM, H, DH = 4, 2048, 1024, 16, 64
HPC = 8            # heads per core
HD = HPC * DH      # 512 head dims per core
NCORES = 8
BF16 = ml_dtypes.bfloat16

TOK_TILE = 128     # k-token tile (partition dim of S^T)
QCHUNK = 512       # q-token chunk (free dim of S^T)
NKT = N // TOK_TILE       # 16 k tiles
NQC = N // QCHUNK         # 4 q chunks
NQT = N // 128            # 16 q tiles (out-projection)
DCH = DIM // 128          # 8 contraction chunks over DIM
VROW = HPC * (DH + 1)     # 520: V_aug row elems per k-tile

_CACHE = {}


def _build_program():
    from contextlib import ExitStack
    import concourse.bass as bass
    import concourse.tile as tile
    from concourse import bacc, mybir

    dt = mybir.dt
    f32 = dt.float32
    bf16 = dt.bfloat16
    Exp = mybir.ActivationFunctionType.Exp

    nc = bacc.Bacc("TRN2", target_bir_lowering=False, debug=False,
                   enable_asserts=False, num_devices=NCORES)

    xT = nc.dram_tensor("xT", [DIM, N], bf16, kind="ExternalInput").ap()
    wq = nc.dram_tensor("wq", [DIM, HD], bf16, kind="ExternalInput").ap()
    wk = nc.dram_tensor("wk", [DIM, HD], bf16, kind="ExternalInput").ap()
    wv = nc.dram_tensor("wv", [DIM, HD], bf16, kind="ExternalInput").ap()
    wo = nc.dram_tensor("wo", [HD, DIM], bf16, kind="ExternalInput").ap()
    kpm = nc.dram_tensor("kpm", [N, 1], f32, kind="ExternalInput").ap()
    cmask_d = nc.dram_tensor("cmask", [4 * 128, QCHUNK], bf16,
                             kind="ExternalInput").ap()
    out_d = nc.dram_tensor("out", [N, DIM], bf16, kind="ExternalOutput").ap()

    with tile.TileContext(nc) as tc, ExitStack() as ctx:
        const = ctx.enter_context(tc.tile_pool(name="const", bufs=1))
        p_sbp = ctx.enter_context(tc.tile_pool(name="p_sbp", bufs=6))
        miscp = ctx.enter_context(tc.tile_pool(name="miscp", bufs=3))
        outp = ctx.enter_context(tc.tile_pool(name="outp", bufs=3))
        mm_ps = ctx.enter_context(tc.tile_pool(name="mm_ps", bufs=2, space="PSUM"))
        s_ps = ctx.enter_context(tc.tile_pool(name="s_ps", bufs=2, space="PSUM"))
        o_ps = ctx.enter_context(tc.tile_pool(name="o_ps", bufs=2, space="PSUM"))

        # ---- persistent SBUF tensors (inputs merged into single tiles so
        # each loads with ONE strided DMA descriptor — the Sync engine
        # issues descriptors at only ~1.6/us, so descriptor count gates
        # the startup) ----
        XT = const.tile([128, DCH * N], bf16, name="XTsb")
        WQ = const.tile([128, DCH * HD], bf16, name="WQsb")
        WK = const.tile([128, DCH * HD], bf16, name="WKsb")
        WV = const.tile([128, DCH * HD], bf16, name="WVsb")
        WO = const.tile([128, 4 * DIM], bf16, name="WOsb")
        xT_sb = [XT[:, c * N:(c + 1) * N] for c in range(DCH)]
        wq_sb = [WQ[:, c * HD:(c + 1) * HD] for c in range(DCH)]
        wk_sb = [WK[:, c * HD:(c + 1) * HD] for c in range(DCH)]
        wv_sb = [WV[:, c * HD:(c + 1) * HD] for c in range(DCH)]
        wo_sb = [WO[:, c * DIM:(c + 1) * DIM] for c in range(4)]
        # Q^T / K^T packed: chunk c holds heads 2c (parts 0-63) and 2c+1 (64-127)
        QT = [const.tile([128, N], bf16, name=f"QTsb{c}") for c in range(4)]
        KT = [const.tile([128, N], bf16, name=f"KTsb{c}") for c in range(4)]
        # V_aug: per k-tile block of 8*(64+1) cols
        V = const.tile([128, NKT * VROW], bf16, name="Vsb")
        # O^T packed like QT/KT
        OT = [const.tile([128, N], bf16, name=f"OTsb{c}") for c in range(4)]
        cmask = const.tile([128, 4 * QCHUNK], bf16, name="cmasksb")
        # key-padding mask: col t = mask[t*128 + p] (one tiny DMA, loaded
        # first so V-proj evacuations never wait behind the big weight loads)
        kpm_sb = const.tile([128, NKT], f32, name="kpmsb")

        sync = nc.sync
        sync.dma_start(
            kpm_sb.rearrange("p (t one) -> p t one", one=1),
            kpm.rearrange("(t p) one -> p t one", p=128),
        )

        # ---- input loads, ordered to match compute order: the first v_proj
        # tiles gate on wv + the first xT column slab; wq/wk land before the
        # PE reaches qk_proj(0); the xT tail lands before v_proj(kt 8-15).
        xT_src = xT.rearrange("(c p) n -> p c n", p=128)
        XT3 = XT.rearrange("p (c n) -> p c n", n=N)
        wv_src = wv.rearrange("(c p) h -> p c h", p=128)
        WV3 = WV.rearrange("p (c h) -> p c h", h=HD)
        sync.dma_start(WV3[:, 0:DCH], wv_src[:, 0:DCH])
        sync.dma_start(XT3[:, :, 0:256], xT_src[:, :, 0:256])
        sync.dma_start(XT3[:, :, 256:1024], xT_src[:, :, 256:1024])
        sync.dma_start(WQ.rearrange("p (c h) -> p c h", h=HD),
                       wq.rearrange("(c p) h -> p c h", p=128))
        sync.dma_start(WK.rearrange("p (c h) -> p c h", h=HD),
                       wk.rearrange("(c p) h -> p c h", p=128))
        sync.dma_start(XT3[:, :, 1024:N], xT_src[:, :, 1024:N])
        # cmask DRAM row r*128+k, col q  ->  SBUF part k, col r*512+q
        sync.dma_start(
            cmask.rearrange("p (r q) -> p r q", r=4),
            cmask_d.rearrange("(r p) q -> p r q", p=128),
        )
        sync.dma_start(WO.rearrange("p (c d) -> p c d", d=DIM),
                       wo.rearrange("(c p) d -> p c d", p=128))

        # ---- V projection: V[tok, dh] via x-stationary matmuls ----
        def v_proj(half):
            for kt in range(half * (NKT // 2), (half + 1) * (NKT // 2)):
                kpm_t = kpm_sb[:, kt:kt + 1]
                ps = mm_ps.tile([128, 512], f32, tag="mm", name="ps")
                for c in range(DCH):
                    nc.tensor.matmul(
                        ps[:], xT_sb[c][:, kt * 128:(kt + 1) * 128],
                        wv_sb[c][:],
                        start=(c == 0), stop=(c == DCH - 1))
                vblk = V[:, kt * VROW:(kt + 1) * VROW].rearrange(
                    "p (h c) -> p h c", c=DH + 1)
                # data cols, scaled by key-padding mask
                nc.vector.tensor_scalar_mul(
                    vblk[:, :, 0:DH],
                    ps.rearrange("p (h c) -> p h c", c=DH),
                    kpm_t[:, 0:1])
                # ones column = mask value (free-dim stride-0 broadcast read)
                nc.vector.tensor_copy(vblk[:, :, DH:DH + 1].squeeze(),
                                      kpm_t[:, 0:1].broadcast_to([128, HPC]))

        def qk_dense(c, tcxs):
            for tcx in tcxs:
                tsl = slice(tcx * QCHUNK, (tcx + 1) * QCHUNK)
                psq = mm_ps.tile([128, 512], f32, tag="mm", name="psq")
                for d in range(DCH):
                    nc.tensor.matmul(
                        psq[:], wq_sb[d][:, c * 128:(c + 1) * 128],
                        xT_sb[d][:, tsl],
                        start=(d == 0), stop=(d == DCH - 1))
                nc.vector.tensor_copy(QT[c][:, tsl], psq[:])
                psk = mm_ps.tile([128, 512], f32, tag="mm", name="psk")
                for d in range(DCH):
                    nc.tensor.matmul(
                        psk[:], wk_sb[d][:, c * 128:(c + 1) * 128],
                        xT_sb[d][:, tsl],
                        start=(d == 0), stop=(d == DCH - 1))
                nc.vector.tensor_copy(KT[c][:, tsl], psk[:])

        # ---- filler machinery: a queue of thunks, each emitting ONE
        # projection / out-projection matmul (plus its PSUM evacuation on the
        # group's last member).  Popped between attend matmuls so the PE has
        # independent work while the Scalar engine runs exp. ----
        filler_q = []
        qk_pending = [0]   # count of qk-proj fillers still queued (FIFO head)

        def pop_filler(n=1):
            for _ in range(min(n, len(filler_q))):
                filler_q.pop(0)()
                if qk_pending[0]:
                    qk_pending[0] -= 1

        def flush_fillers():
            pop_filler(len(filler_q))

        def flush_qk_fillers():
            pop_filler(qk_pending[0])

        def make_qk_fillers(c, tcxs):
            thunks = []
            for tcx in tcxs:
                tsl = slice(tcx * QCHUNK, (tcx + 1) * QCHUNK)
                for wsb, dst in ((wq_sb, QT), (wk_sb, KT)):
                    cell = {}
                    for d in range(DCH):
                        def emit(d=d, wsb=wsb, dst=dst, tsl=tsl, cell=cell,
                                 c=c):
                            if d == 0:
                                cell["ps"] = mm_ps.tile([128, 512], f32,
                                                        tag="mm", name="psf")
                            nc.tensor.matmul(
                                cell["ps"][:],
                                wsb[d][:, c * 128:(c + 1) * 128],
                                xT_sb[d][:, tsl],
                                start=(d == 0), stop=(d == DCH - 1),
                                skip_group_check=True)
                            if d == DCH - 1:
                                nc.vector.tensor_copy(dst[c][:, tsl],
                                                      cell["ps"][:])
                        thunks.append(emit)
            return thunks

        def make_out_fillers(qts, tail=False):
            # NOTE: an all-per-qt (full-row) variant of the mid-phase output
            # writes compiled into a binary that ran with ALL engines ~20%
            # slower (repeatable; the state is fixed at load time, set
            # before any output DMA runs). Keep mid-phase writes per-oc;
            # only the tail groups use full rows for a faster drain.
            thunks = []
            for qt in qts:
                cell = {}
                for oc in range(2):
                    for cc in range(4):
                        def emit(qt=qt, oc=oc, cc=cc, cell=cell, tail=tail):
                            if oc == 0 and cc == 0:
                                cell["y"] = outp.tile([128, DIM], bf16,
                                                      tag="y", name="y_sb")
                            if cc == 0:
                                cell["ps"] = mm_ps.tile([128, 512], f32,
                                                        tag="mm", name="psy")
                            nc.tensor.matmul(
                                cell["ps"][:], OT[cc][:, qt * 128:(qt + 1) * 128],
                                wo_sb[cc][:, oc * 512:(oc + 1) * 512],
                                start=(cc == 0), stop=(cc == 3),
                                skip_group_check=True)
                            if cc == 3:
                                y = cell["y"]
                                ysl = y[:, oc * 512:(oc + 1) * 512]
                                # in the tail the Scalar engine is idle (no
                                # exps left), so evacuate there and keep DVE
                                # free for the normalize chain
                                if tail:
                                    nc.scalar.copy(ysl, cell["ps"][:])
                                else:
                                    nc.vector.tensor_copy(ysl, cell["ps"][:])
                                    sync.dma_start(
                                        out_d[qt * 128:(qt + 1) * 128,
                                              oc * 512:(oc + 1) * 512], ysl)
                                if tail and oc == 1:
                                    sync.dma_start(
                                        out_d[qt * 128:(qt + 1) * 128, :],
                                        y[:])
                        thunks.append(emit)
            return thunks

        # ---- attend, software-pipelined: scores of k-pair kp+1 are issued
        # before PV of kp, and fillers slot between them, so the PE never
        # waits directly on the Scalar engine's exp of kp. ----
        def attend(h, qc, pops_per_kp=1):
            c = h // 2
            po = (h % 2) * 64          # partition offset within chunk
            qt_h = QT[c][po:po + 64, :]
            kt_h = KT[c][po:po + 64, :]
            qsl = slice(qc * QCHUNK, (qc + 1) * QCHUNK)
            pso = o_ps.tile([DH + 1, 512], f32, tag="o", name="pso")
            nkt = 4 * qc + 4
            kps = nkt // 2

            def emit_scores_exp(kp):
                ps2 = s_ps.tile([128, 1024], f32, tag="s", name="ps2")
                r = 2 * kp - 4 * qc
                for j in (0, 1):
                    kt = 2 * kp + j
                    # diagonal k-tile: q < (r+j)*128 fully masked -> narrow
                    off = max(0, (kt - 4 * qc) * 128)
                    nc.tensor.matmul(
                        ps2[:, j * 512 + off:(j + 1) * 512],
                        kt_h[:, kt * 128:(kt + 1) * 128],
                        qt_h[:, qc * QCHUNK + off:(qc + 1) * QCHUNK],
                        start=True, stop=True)
                p2 = p_sbp.tile([128, 1024], bf16, tag="p", name="p2")
                if r >= 0:
                    # per-half exp + causal mask over only the written cols
                    for j in (0, 1):
                        off = (r + j) * 128
                        sl = slice(j * 512 + off, (j + 1) * 512)
                        nc.scalar.activation(p2[:, sl], ps2[:, sl], Exp)
                        nc.vector.tensor_mul(
                            p2[:, sl], p2[:, sl],
                            cmask[:, (r + j) * QCHUNK + off:
                                  (r + j + 1) * QCHUNK])
                else:
                    nc.scalar.activation(p2[:], ps2[:], Exp)
                return p2

            def emit_pv(kp, p2):
                for j in (0, 1):
                    kt = 2 * kp + j
                    off = max(0, (kt - 4 * qc) * 128)
                    nc.tensor.matmul(
                        pso[:, off:512],
                        V[:, kt * VROW + h * (DH + 1):
                           kt * VROW + (h + 1) * (DH + 1)],
                        p2[:, j * 512 + off:(j + 1) * 512],
                        start=(kt == 0), stop=(kt == nkt - 1),
                        skip_group_check=True)

            pop_filler(2)
            prev = emit_scores_exp(0)
            for kp in range(1, kps):
                cur = emit_scores_exp(kp)
                pop_filler(pops_per_kp)
                emit_pv(kp - 1, prev)
                prev = cur
            pop_filler(pops_per_kp)
            emit_pv(kps - 1, prev)

            # normalize: O^T[0:64] * (1 / rowsum row 64)
            # (stage rowsum into SBUF: custom-DVE recip needs SBUF in)
            rsum = miscp.tile([1, 512], f32, tag="rsum", name="rsum")
            nc.vector.tensor_copy(rsum[:], pso[DH:DH + 1, :])
            recip = miscp.tile([1, 512], f32, tag="recip", name="recip")
            nc.vector.reciprocal_approx_fast(recip[:], rsum[:])
            bcast = miscp.tile([64, 512], f32, tag="bcast", name="bcast")
            nc.gpsimd.partition_broadcast(bcast[:], recip[:])
            if po == 0:
                nc.vector.tensor_mul(OT[c][0:64, qsl],
                                     pso[0:DH, :], bcast[:])
            else:
                otmp = miscp.tile([64, 512], bf16, tag="otmp", bufs=3,
                                  name="otmp")
                nc.vector.tensor_mul(otmp[:], pso[0:DH, :], bcast[:])
                # partition shift 0->64 needs a DMA, engines can't shift
                sync.dma_start(OT[c][64:128, qsl], otmp[:])

        # ---- startup: V and QK(0) projections, ordered against the DMA
        # stream (v_proj kt 8-15 needs the xT tail, which lands after wq/wk)
        v_proj(0)
        qk_dense(0, [0, 1])
        v_proj(1)
        qk_dense(0, [2, 3])

        # ---- pairs 0-2: heads in sequence, next chunk's qk_proj as fillers
        for cpair in (0, 1, 2):
            if cpair < 2:
                qk_f = make_qk_fillers(cpair + 1, [0, 1, 2, 3])
            else:
                # hold back tcx3 of chunk 3: fillers for the start of pair 3
                qk_f = make_qk_fillers(3, [0, 1, 2])
            filler_q.extend(qk_f)
            qk_pending[0] += len(qk_f)
            for h in (2 * cpair, 2 * cpair + 1):
                for qc in range(NQC):
                    attend(h, qc, pops_per_kp=1)
            flush_fillers()

        # ---- pair 3: heads interleaved per qc so out_proj tiles become
        # ready progressively; out fillers enqueue one step after their
        # normalize chains resolve, so popping them never stalls the PE.
        qk_f = make_qk_fillers(3, [3])
        filler_q.extend(qk_f)
        qk_pending[0] += len(qk_f)
        for qc in range(NQC):
            if qc == 3:
                # qk tcx3 must be fully emitted before the qc=3 attends
                # (reads would otherwise precede writes in program order)
                flush_qk_fillers()
            attend(7, qc, pops_per_kp=2)
            if qc >= 1:
                filler_q.extend(
                    make_out_fillers(range(4 * (qc - 1), 4 * qc)))
            attend(6, qc, pops_per_kp=2)
        # drain: remaining qt 8-11 fillers cover the last normalize chains,
        # then the final qt 12-15 go full-row with Scalar-engine evacuation
        flush_fillers()
        for t in make_out_fillers(range(12, 16), tail=True):
            t()

    nc.compile()
    return nc


def _get_program():
    if "nc" not in _CACHE:
        _CACHE["nc"] = _build_program()
    return _CACHE["nc"]


def _prep_inputs(x, mask, w_qkv, w_out):
    """Build the 8 per-core input maps (host-side sharding)."""
    scale = DH ** -0.5
    # causal keep-mask patterns for the 4 diagonal k-tiles of a 512 q-chunk
    k_idx = np.arange(128)[:, None]
    q_idx = np.arange(QCHUNK)[None, :]
    cm = np.concatenate(
        [(q_idx >= r * 128 + k_idx) for r in range(4)], axis=0
    ).astype(BF16)  # [512, 512]

    xT = [np.ascontiguousarray(x[b].T).astype(BF16) for b in range(B)]
    in_maps = []
    for core in range(NCORES):
        b, hg = core // 2, core % 2
        cs = slice(hg * HD, (hg + 1) * HD)
        wq_s = (w_qkv[:, 0 * DIM:1 * DIM][:, cs] * scale).astype(BF16)
        wk_s = w_qkv[:, 1 * DIM:2 * DIM][:, cs].astype(BF16)
        wv_s = w_qkv[:, 2 * DIM:3 * DIM][:, cs].astype(BF16)
        wo_s = np.ascontiguousarray(w_out[cs, :]).astype(BF16)
        kpm = mask[b].astype(np.float32).reshape(N, 1)
        in_maps.append({
            "xT": xT[b], "wq": wq_s, "wk": wk_s, "wv": wv_s, "wo": wo_s,
            "kpm": np.ascontiguousarray(kpm), "cmask": cm,
        })
    return in_maps


def kernel(x, mask, w_qkv, w_out, b_out, _trace=False):
    from concourse import bass_utils

    x = np.asarray(x, dtype=np.float32)
    mask = np.asarray(mask)
    w_qkv = np.asarray(w_qkv, dtype=np.float32)
    w_out = np.asarray(w_out, dtype=np.float32)
    b_out = np.asarray(b_out, dtype=np.float32)

    nc = _get_program()
    in_maps = _prep_inputs(x, mask, w_qkv, w_out)
    res = bass_utils.run_bass_kernel_spmd(
        nc, in_maps, core_ids=list(range(NCORES)), trace=_trace)

    out = np.empty((B, N, DIM), dtype=np.float32)
    for b in range(B):
        out[b] = (res.results[2 * b]["out"].astype(np.float32)
                  + res.results[2 * b + 1]["out"].astype(np.float32) + b_out)
    if _trace:
        return out, res
    return out


# revision 8
# speedup vs baseline: 1.0314x; 1.0314x over previous
"""Trainium2 Bass kernel for causal multi-head attention (B=4, N=2048, DIM=1024, H=16, DH=64).

Sharding: 8 cores = (batch, head-group) pairs. Core c handles batch c//2 and
heads (c%2)*8 .. (c%2)*8+7.  Each core computes QKV projection for its 8 heads,
causal flash-attention, and a partial output projection (its heads' rows of
w_out).  The host sums the two partial outputs per batch and adds b_out.

Device-side layout choices (per core):
  - x is fed pre-transposed as xT [DIM, N] bf16 (host prep), so the QKV
    projection contraction (over DIM) sits on partitions with no on-device
    transpose.
  - Q^T, K^T computed as [head_dim, tok] (weights-stationary matmuls) so that
    scores can be computed directly as S^T = K^T.T @ Q^T with contraction dh=64.
  - S^T tiles are [128 k-tok, 512 q-tok]; softmax denominator comes free by
    augmenting V with a ones column: O^T_aug = [V | 1].T @ exp(S^T).
  - V computed as [tok, dh] (x-stationary matmuls), stored interleaved with the
    ones column: per k-tile [128, 8*65].
  - Causal masking: multiply exp(S^T) by precomputed 0/1 bf16 tiles on the
    diagonal blocks only (exp of a finite garbage score times 0 is exactly 0).
  - Key-padding mask folds into V_aug: V_aug row k scaled by mask[k] zeroes both
    numerator and denominator contributions of masked keys.

Schedule: the PE is the bottleneck engine (matmul stream floor ~221us).  The
attend loops process a HEAD PAIR interleaved (A/B) and software-pipelined
(scores of k-pair kp+1 issue before PV of kp), which doubles the exp pipeline
depth within the same PSUM budget, so the PE does not sit behind the Scalar
engine's exp.  Projection / out-projection matmuls are woven one-at-a-time
into the attend loops as "fillers" that absorb the residual exp-latency
stalls.  O^T is evacuated UNNORMALIZED (frees the PV PSUM tile early) and
scaled in place on SBUF afterwards.  Evacuations are spread across engines:
qk-filler copies on GpSimd, out-projection copies on Scalar (idle during the
out phase), keeping the Vector engine's mask-multiply latency low.
"""

import numpy as np
import ml_dtypes

B, N, DIM, H, DH = 4, 2048, 1024, 16, 64
HPC = 8            # heads per core
HD = HPC * DH      # 512 head dims per core
NCORES = 8
BF16 = ml_dtypes.bfloat16

TOK_TILE = 128     # k-token tile (partition dim of S^T)
QCHUNK = 512       # q-token chunk (free dim of S^T)
NKT = N // TOK_TILE       # 16 k tiles
NQC = N // QCHUNK         # 4 q chunks
NQT = N // 128            # 16 q tiles (out-projection)
DCH = DIM // 128          # 8 contraction chunks over DIM
VROW = HPC * (DH + 1)     # 520: V_aug row elems per k-tile

_CACHE = {}


def _build_program():
    from contextlib import ExitStack
    import concourse.bass as bass
    import concourse.tile as tile
    from concourse import bacc, mybir

    dt = mybir.dt
    f32 = dt.float32
    bf16 = dt.bfloat16
    Exp = mybir.ActivationFunctionType.Exp

    nc = bacc.Bacc("TRN2", target_bir_lowering=False, debug=False,
                   enable_asserts=False, num_devices=NCORES)

    xT = nc.dram_tensor("xT", [DIM, N], bf16, kind="ExternalInput").ap()
    wq = nc.dram_tensor("wq", [DIM, HD], bf16, kind="ExternalInput").ap()
    wk = nc.dram_tensor("wk", [DIM, HD], bf16, kind="ExternalInput").ap()
    wv = nc.dram_tensor("wv", [DIM, HD], bf16, kind="ExternalInput").ap()
    wo = nc.dram_tensor("wo", [HD, DIM], bf16, kind="ExternalInput").ap()
    kpm = nc.dram_tensor("kpm", [N, 1], f32, kind="ExternalInput").ap()
    cmask_d = nc.dram_tensor("cmask", [4 * 128, QCHUNK], bf16,
                             kind="ExternalInput").ap()
    out_d = nc.dram_tensor("out", [N, DIM], bf16, kind="ExternalOutput").ap()

    with tile.TileContext(nc) as tc, ExitStack() as ctx:
        const = ctx.enter_context(tc.tile_pool(name="const", bufs=1))
        p_sbp = ctx.enter_context(tc.tile_pool(name="p_sbp", bufs=6))
        miscp = ctx.enter_context(tc.tile_pool(name="miscp", bufs=3))
        outp = ctx.enter_context(tc.tile_pool(name="outp", bufs=3))
        mm_ps = ctx.enter_context(tc.tile_pool(name="mm_ps", bufs=2, space="PSUM"))
        s_ps = ctx.enter_context(tc.tile_pool(name="s_ps", bufs=2, space="PSUM"))
        o_ps = ctx.enter_context(tc.tile_pool(name="o_ps", bufs=2, space="PSUM"))

        # ---- persistent SBUF tensors (inputs merged into single tiles so
        # each loads with ONE strided DMA descriptor — the Sync engine
        # issues descriptors at only ~1.6/us, so descriptor count gates
        # the startup) ----
        XT = const.tile([128, DCH * N], bf16, name="XTsb")
        WQ = const.tile([128, DCH * HD], bf16, name="WQsb")
        WK = const.tile([128, DCH * HD], bf16, name="WKsb")
        WV = const.tile([128, DCH * HD], bf16, name="WVsb")
        WO = const.tile([128, 4 * DIM], bf16, name="WOsb")
        xT_sb = [XT[:, c * N:(c + 1) * N] for c in range(DCH)]
        wq_sb = [WQ[:, c * HD:(c + 1) * HD] for c in range(DCH)]
        wk_sb = [WK[:, c * HD:(c + 1) * HD] for c in range(DCH)]
        wv_sb = [WV[:, c * HD:(c + 1) * HD] for c in range(DCH)]
        wo_sb = [WO[:, c * DIM:(c + 1) * DIM] for c in range(4)]
        # Q^T / K^T packed: chunk c holds heads 2c (parts 0-63) and 2c+1 (64-127)
        QT = [const.tile([128, N], bf16, name=f"QTsb{c}") for c in range(4)]
        KT = [const.tile([128, N], bf16, name=f"KTsb{c}") for c in range(4)]
        # V_aug: per k-tile block of 8*(64+1) cols
        V = const.tile([128, NKT * VROW], bf16, name="Vsb")
        # O^T packed like QT/KT
        OT = [const.tile([128, N], bf16, name=f"OTsb{c}") for c in range(4)]
        cmask = const.tile([128, 4 * QCHUNK], bf16, name="cmasksb")
        # key-padding mask: col t = mask[t*128 + p] (one tiny DMA, loaded
        # first so V-proj evacuations never wait behind the big weight loads)
        kpm_sb = const.tile([128, NKT], f32, name="kpmsb")

        sync = nc.sync
        sync.dma_start(
            kpm_sb.rearrange("p (t one) -> p t one", one=1),
            kpm.rearrange("(t p) one -> p t one", p=128),
        )

        # ---- load inputs: wv + xT first 256 cols first so the first v_proj
        # accumulation groups start early; each pass is ONE descriptor ----
        xT_src = xT.rearrange("(c p) n -> p c n", p=128)
        XT3 = XT.rearrange("p (c n) -> p c n", n=N)
        wv_src = wv.rearrange("(c p) h -> p c h", p=128)
        WV3 = WV.rearrange("p (c h) -> p c h", h=HD)
        # finer granularity on the critical first pieces so the first v_proj
        # accumulation group streams in as chunks land (it contracts chunks
        # in order, so chunk 0-1 arriving first lets the group start)
        sync.dma_start(WV3[:, 0:4], wv_src[:, 0:4])
        sync.dma_start(XT3[:, 0:4, 0:256], xT_src[:, 0:4, 0:256])
        sync.dma_start(WV3[:, 4:8], wv_src[:, 4:8])
        sync.dma_start(XT3[:, 4:8, 0:256], xT_src[:, 4:8, 0:256])
        sync.dma_start(XT3[:, :, 256:N // 2], xT_src[:, :, 256:N // 2])
        sync.dma_start(XT3[:, :, N // 2:N], xT_src[:, :, N // 2:N])
        sync.dma_start(WQ.rearrange("p (c h) -> p c h", h=HD),
                       wq.rearrange("(c p) h -> p c h", p=128))
        sync.dma_start(WK.rearrange("p (c h) -> p c h", h=HD),
                       wk.rearrange("(c p) h -> p c h", p=128))
        sync.dma_start(WO.rearrange("p (c d) -> p c d", d=DIM),
                       wo.rearrange("(c p) d -> p c d", p=128))
        # cmask DRAM row r*128+k, col q  ->  SBUF part k, col r*512+q
        sync.dma_start(
            cmask.rearrange("p (r q) -> p r q", r=4),
            cmask_d.rearrange("(r p) q -> p r q", p=128),
        )

        # ---- V projection: V[tok, dh] via x-stationary matmuls ----
        def v_proj(half):
            for kt in range(half * (NKT // 2), (half + 1) * (NKT // 2)):
                kpm_t = kpm_sb[:, kt:kt + 1]
                ps = mm_ps.tile([128, 512], f32, tag="mm", name="ps")
                for c in range(DCH):
                    nc.tensor.matmul(
                        ps[:], xT_sb[c][:, kt * 128:(kt + 1) * 128],
                        wv_sb[c][:],
                        start=(c == 0), stop=(c == DCH - 1))
                vblk = V[:, kt * VROW:(kt + 1) * VROW].rearrange(
                    "p (h c) -> p h c", c=DH + 1)
                # data cols, scaled by key-padding mask
                nc.vector.tensor_scalar_mul(
                    vblk[:, :, 0:DH],
                    ps.rearrange("p (h c) -> p h c", c=DH),
                    kpm_t[:, 0:1])
                # ones column = mask value (free-dim stride-0 broadcast read)
                nc.vector.tensor_copy(vblk[:, :, DH:DH + 1].squeeze(),
                                      kpm_t[:, 0:1].broadcast_to([128, HPC]))

        def qk_dense(c):
            for tcx in range(NQC):
                tsl = slice(tcx * QCHUNK, (tcx + 1) * QCHUNK)
                psq = mm_ps.tile([128, 512], f32, tag="mm", name="psq")
                for d in range(DCH):
                    nc.tensor.matmul(
                        psq[:], wq_sb[d][:, c * 128:(c + 1) * 128],
                        xT_sb[d][:, tsl],
                        start=(d == 0), stop=(d == DCH - 1))
                nc.vector.tensor_copy(QT[c][:, tsl], psq[:])
                psk = mm_ps.tile([128, 512], f32, tag="mm", name="psk")
                for d in range(DCH):
                    nc.tensor.matmul(
                        psk[:], wk_sb[d][:, c * 128:(c + 1) * 128],
                        xT_sb[d][:, tsl],
                        start=(d == 0), stop=(d == DCH - 1))
                nc.vector.tensor_copy(KT[c][:, tsl], psk[:])

        # ---- filler machinery: a queue of thunks, each emitting ONE
        # projection / out-projection matmul (plus its PSUM evacuation on the
        # group's last member).  Popped between attend matmuls so the PE has
        # independent work while the Scalar engine runs exp. ----
        filler_q = []
        qk_pending = [0]   # count of qk-proj fillers still queued (FIFO head)

        def pop_filler(n=1):
            for _ in range(min(n, len(filler_q))):
                filler_q.pop(0)()
                if qk_pending[0]:
                    qk_pending[0] -= 1

        def flush_fillers():
            pop_filler(len(filler_q))

        def flush_qk_fillers():
            pop_filler(qk_pending[0])

        def make_qk_fillers(c, tcxs):
            thunks = []
            for tcx in tcxs:
                tsl = slice(tcx * QCHUNK, (tcx + 1) * QCHUNK)
                for wsb, dst in ((wq_sb, QT), (wk_sb, KT)):
                    cell = {}
                    for d in range(DCH):
                        def emit(d=d, wsb=wsb, dst=dst, tsl=tsl, cell=cell,
                                 c=c):
                            if d == 0:
                                cell["ps"] = mm_ps.tile([128, 512], f32,
                                                        tag="mm", name="psf")
                            nc.tensor.matmul(
                                cell["ps"][:],
                                wsb[d][:, c * 128:(c + 1) * 128],
                                xT_sb[d][:, tsl],
                                start=(d == 0), stop=(d == DCH - 1),
                                skip_group_check=True)
                            if d == DCH - 1:
                                nc.vector.tensor_copy(dst[c][:, tsl],
                                                      cell["ps"][:])
                        thunks.append(emit)
            return thunks

        def make_out_fillers(qts, tail=False):
            # NOTE: an all-per-qt (full-row) variant of the mid-phase output
            # writes compiled into a binary that ran with ALL engines ~20%
            # slower (repeatable; the state is fixed at load time, set
            # before any output DMA runs). Keep mid-phase writes per-oc;
            # only the tail groups use full rows for a faster drain.
            thunks = []
            for qt in qts:
                cell = {}
                for oc in range(2):
                    for cc in range(4):
                        def emit(qt=qt, oc=oc, cc=cc, cell=cell, tail=tail):
                            if oc == 0 and cc == 0:
                                cell["y"] = outp.tile([128, DIM], bf16,
                                                      tag="y", name="y_sb")
                            if cc == 0:
                                cell["ps"] = mm_ps.tile([128, 512], f32,
                                                        tag="mm", name="psy")
                            nc.tensor.matmul(
                                cell["ps"][:], OT[cc][:, qt * 128:(qt + 1) * 128],
                                wo_sb[cc][:, oc * 512:(oc + 1) * 512],
                                start=(cc == 0), stop=(cc == 3),
                                skip_group_check=True)
                            if cc == 3:
                                y = cell["y"]
                                ysl = y[:, oc * 512:(oc + 1) * 512]
                                # evacuate on the Scalar engine: it is idle
                                # during the out phase (no exps there)
                                nc.scalar.copy(ysl, cell["ps"][:])
                                if not tail:
                                    sync.dma_start(
                                        out_d[qt * 128:(qt + 1) * 128,
                                              oc * 512:(oc + 1) * 512], ysl)
                                elif oc == 1:
                                    sync.dma_start(
                                        out_d[qt * 128:(qt + 1) * 128, :],
                                        y[:])
                        thunks.append(emit)
            return thunks

        # ---- attend for a HEAD PAIR, A/B-interleaved and software-
        # pipelined: scores of k-pair kp+1 (both heads) issue before PV of
        # kp, giving the Scalar engine ~2 full k-pair periods to finish each
        # exp before its PV needs it.  Fillers slot between the matmuls. ----
        def attend_pair(ha, hb, qc):
            c = ha // 2
            qsl = slice(qc * QCHUNK, (qc + 1) * QCHUNK)
            nkt = 4 * qc + 4
            kps = nkt // 2
            heads = (ha, hb)
            pos = {h: (h % 2) * 64 for h in heads}
            pso = {h: o_ps.tile([DH + 1, 512], f32, tag="o", name="pso")
                   for h in heads}

            def emit_scores_exp(h, kp):
                po = pos[h]
                qt_h = QT[c][po:po + 64, :]
                kt_h = KT[c][po:po + 64, :]
                ps2 = s_ps.tile([128, 1024], f32, tag="s", name="ps2")
                r = 2 * kp - 4 * qc
                for j in (0, 1):
                    kt = 2 * kp + j
                    # diagonal k-tile: q < (r+j)*128 fully masked -> narrow
                    off = max(0, (kt - 4 * qc) * 128)
                    nc.tensor.matmul(
                        ps2[:, j * 512 + off:(j + 1) * 512],
                        kt_h[:, kt * 128:(kt + 1) * 128],
                        qt_h[:, qc * QCHUNK + off:(qc + 1) * QCHUNK],
                        start=True, stop=True)
                p2 = p_sbp.tile([128, 1024], bf16, tag="p", name="p2")
                if r >= 0:
                    # per-half exp + causal mask over only the written cols
                    for j in (0, 1):
                        off = (r + j) * 128
                        sl = slice(j * 512 + off, (j + 1) * 512)
                        nc.scalar.activation(p2[:, sl], ps2[:, sl], Exp)
                        nc.vector.tensor_mul(
                            p2[:, sl], p2[:, sl],
                            cmask[:, (r + j) * QCHUNK + off:
                                  (r + j + 1) * QCHUNK])
                else:
                    nc.scalar.activation(p2[:], ps2[:], Exp)
                return p2

            def emit_pv(h, kp, p2):
                for j in (0, 1):
                    kt = 2 * kp + j
                    off = max(0, (kt - 4 * qc) * 128)
                    nc.tensor.matmul(
                        pso[h][:, off:512],
                        V[:, kt * VROW + h * (DH + 1):
                           kt * VROW + (h + 1) * (DH + 1)],
                        p2[:, j * 512 + off:(j + 1) * 512],
                        start=(kt == 0), stop=(kt == nkt - 1),
                        skip_group_check=True)

            pop_filler(2)
            prev = {ha: emit_scores_exp(ha, 0), hb: emit_scores_exp(hb, 0)}
            for kp in range(1, kps):
                cur_a = emit_scores_exp(ha, kp)
                cur_b = emit_scores_exp(hb, kp)
                pop_filler(1)
                emit_pv(ha, kp - 1, prev[ha])
                pop_filler(1)
                emit_pv(hb, kp - 1, prev[hb])
                prev = {ha: cur_a, hb: cur_b}
            pop_filler(1)
            emit_pv(ha, kps - 1, prev[ha])
            pop_filler(1)
            emit_pv(hb, kps - 1, prev[hb])

            # normalize, with early PSUM release: evacuate O^T UNNORMALIZED
            # (cast to bf16; the unnormalized magnitudes are ~1e2, well in
            # bf16 range), then scale in place on SBUF once 1/rowsum is
            # broadcast.  pso frees after two copies instead of the full
            # recip chain.
            for h in heads:
                po = pos[h]
                rsum = miscp.tile([1, 512], f32, tag="rsum", name="rsum")
                nc.vector.tensor_copy(rsum[:], pso[h][DH:DH + 1, :])
                if po == 0:
                    dst = OT[c][0:64, qsl]
                    nc.vector.tensor_copy(dst, pso[h][0:DH, :])
                else:
                    otmp = miscp.tile([64, 512], bf16, tag="otmp", bufs=3,
                                      name="otmp")
                    dst = otmp[:]
                    nc.vector.tensor_copy(dst, pso[h][0:DH, :])
                recip = miscp.tile([1, 512], f32, tag="recip", name="recip")
                nc.vector.reciprocal_approx_fast(recip[:], rsum[:])
                bcast = miscp.tile([64, 512], f32, tag="bcast", name="bcast")
                nc.gpsimd.partition_broadcast(bcast[:], recip[:])
                nc.vector.tensor_mul(dst, dst, bcast[:])
                if po != 0:
                    # partition shift 0->64 needs a DMA, engines can't shift
                    sync.dma_start(OT[c][64:128, qsl], dst)

        # ---- startup: V and QK(0) projections (dense; DMA-gated anyway)
        v_proj(0)
        v_proj(1)
        qk_dense(0)

        # ---- pairs 0-2: next chunk's qk_proj matmuls as fillers
        for cpair in (0, 1, 2):
            if cpair < 2:
                qk_f = make_qk_fillers(cpair + 1, [0, 1, 2, 3])
            else:
                # hold back tcx3 of chunk 3: fillers for the start of pair 3
                qk_f = make_qk_fillers(3, [0, 1, 2])
            filler_q.extend(qk_f)
            qk_pending[0] += len(qk_f)
            for qc in range(NQC):
                attend_pair(2 * cpair, 2 * cpair + 1, qc)
            flush_fillers()

        # ---- pair 3: out-projection fillers become ready progressively
        # (qt group g needs all heads' normalized OT at qc=g); enqueue each
        # group one qc step after its chains resolve so pops never stall.
        qk_f = make_qk_fillers(3, [3])
        filler_q.extend(qk_f)
        qk_pending[0] += len(qk_f)
        for qc in range(NQC):
            if qc == 3:
                # qk tcx3 must be fully emitted before the qc=3 attends
                flush_qk_fillers()
            attend_pair(7, 6, qc)
            if qc >= 1:
                filler_q.extend(
                    make_out_fillers(range(4 * (qc - 1), 4 * qc)))
        # drain: remaining qt 8-11 fillers cover the last normalize chains,
        # then the final qt 12-15 go full-row with Scalar-engine evacuation
        flush_fillers()
        for t in make_out_fillers(range(12, 16), tail=True):
            t()

    nc.compile()
    return nc


def _get_program():
    if "nc" not in _CACHE:
        _CACHE["nc"] = _build_program()
    return _CACHE["nc"]


def _prep_inputs(x, mask, w_qkv, w_out):
    """Build the 8 per-core input maps (host-side sharding)."""
    scale = DH ** -0.5
    # causal keep-mask patterns for the 4 diagonal k-tiles of a 512 q-chunk
    k_idx = np.arange(128)[:, None]
    q_idx = np.arange(QCHUNK)[None, :]
    cm = np.concatenate(
        [(q_idx >= r * 128 + k_idx) for r in range(4)], axis=0
    ).astype(BF16)  # [512, 512]

    xT = [np.ascontiguousarray(x[b].T).astype(BF16) for b in range(B)]
    in_maps = []
    for core in range(NCORES):
        b, hg = core // 2, core % 2
        cs = slice(hg * HD, (hg + 1) * HD)
        wq_s = (w_qkv[:, 0 * DIM:1 * DIM][:, cs] * scale).astype(BF16)
        wk_s = w_qkv[:, 1 * DIM:2 * DIM][:, cs].astype(BF16)
        wv_s = w_qkv[:, 2 * DIM:3 * DIM][:, cs].astype(BF16)
        wo_s = np.ascontiguousarray(w_out[cs, :]).astype(BF16)
        kpm = mask[b].astype(np.float32).reshape(N, 1)
        in_maps.append({
            "xT": xT[b], "wq": wq_s, "wk": wk_s, "wv": wv_s, "wo": wo_s,
            "kpm": np.ascontiguousarray(kpm), "cmask": cm,
        })
    return in_maps


def kernel(x, mask, w_qkv, w_out, b_out, _trace=False):
    from concourse import bass_utils

    x = np.asarray(x, dtype=np.float32)
    mask = np.asarray(mask)
    w_qkv = np.asarray(w_qkv, dtype=np.float32)
    w_out = np.asarray(w_out, dtype=np.float32)
    b_out = np.asarray(b_out, dtype=np.float32)

    nc = _get_program()
    in_maps = _prep_inputs(x, mask, w_qkv, w_out)
    res = bass_utils.run_bass_kernel_spmd(
        nc, in_maps, core_ids=list(range(NCORES)), trace=_trace)

    out = np.empty((B, N, DIM), dtype=np.float32)
    for b in range(B):
        out[b] = (res.results[2 * b]["out"].astype(np.float32)
                  + res.results[2 * b + 1]["out"].astype(np.float32) + b_out)
    if _trace:
        return out, res
    return out
